# revision 46
# baseline (speedup 1.0000x reference)
"""3-layer GCN (GCNConv + residual + relu, global add pool, MLP softmax) on 8
Trainium2 NeuronCores.

Sharding: nodes/edges partitioned by destination-node range across the 8
cores. Per layer: each core computes its shard of xw' = (D^-1/2 h) @ Wg
(bf16, node-major via PE transposes, PSUM copies drained on the Act engine)
and ONE AllGather per layer moves the full message table to every core
(single large collective: the cost model rewards size; gather chunk tables
are row-interleaved strided views, chunk = row %% 4, idx = row // 4, so each
chunk's dma_gather index space fits int16). Phase B dma_gathers per-edge
message rows and segment-sums them into per-128-dst-window PSUM accumulators
via one-hot matmuls; the fp16 one-hots are built on the DVE at its 2x rate
(packed fp16 operands: dstrel slabs against a host-built "staircase"
constant, transposed [128, dst, tile] layout so every operand has a packed
inner dim; window-crossing extras use host-prepared shifted dstrel columns).
The next layer's phase A + AllGather are emitted right after the last
epilogue so the collective overlaps the layer tail; the last layer folds the
global-add-pool matmuls into the epilogues. Pooled [64,128] partials are
AllGathered and summed locally (cheaper than AllReduce in the collective
model) and the tiny classifier is replicated.
All cores run the IDENTICAL program; per-core variation lives entirely in
data (gather indices, sel values, padding).
"""
import math
import numpy as np
import ml_dtypes

import concourse.bacc as bacc
import concourse.bass as bass
import concourse.mybir as mybir
import concourse.tile as tile
from concourse.bass_utils import run_bass_kernel_spmd

NCORES = 8
G = 64    # graphs in batch (pooled rows)
C = 2     # classes
SBW = 8   # dst windows per superblock (psum granularity)
SLAB = 32  # sel tiles built per is_equal op

bf16 = ml_dtypes.bfloat16
_cache = {}


def _ceil(a, b):
    return -(-a // b)


def _ceilarr(a, b):
    return -(-a // b)


# --------------------------------------------------------------------------
# host preprocessing
# --------------------------------------------------------------------------
def _preprocess(x, edge_index, batch):
    N, D = x.shape
    assert D == 128 and N % NCORES == 0
    NLOC = N // NCORES
    NPAD = _ceil(NLOC, 128) * 128
    NW = NPAD // 128
    NSB = _ceil(NW, SBW)

    # single AllGather per layer into T [NCORES*128, NW*128]; gather chunk
    # tables are row-interleaved views (chunk = row % 4, idx = row // 4) so
    # each chunk's index space fits int16
    TROWS = NCORES * 128 * NW
    CHR = TROWS // 4
    assert CHR <= 32768 and TROWS % 4 == 0

    src = np.asarray(edge_index[0], np.int64)
    dst = np.asarray(edge_index[1], np.int64)
    deg = np.bincount(dst, minlength=N).astype(np.float64) + 1.0
    dinv = (deg ** -0.5).astype(np.float32)

    loops = np.arange(N, dtype=np.int64)
    src_f = np.concatenate([src, loops])
    dst_f = np.concatenate([dst, loops])

    core = dst_f // NLOC
    dl = dst_f - core * NLOC
    sc = src_f // NLOC
    sl = src_f - sc * NLOC
    p_s = sl % 128
    a_s = sl // 128
    r_g = (sc * 128 + p_s) * NW + a_s        # global row in T
    ch = r_g % 4
    srow = r_g // 4                          # gather idx within chunk view
    w = dl // 128
    cell = w * 4 + ch                        # per-core cell id

    key = core * (NW * 4) + cell
    counts = np.bincount(key, minlength=NCORES * NW * 4).reshape(NCORES, NW * 4)
    import os
    _cgran = int(os.environ.get("CELL_GRAN", "8"))
    cmax = counts.max(axis=0)                # [NW*4]
    cap = _ceilarr(cmax, _cgran) * _cgran    # slots per cell (0 if empty)

    # stream layout: sb -> ch -> w; groups padded to x128
    cell_slot_off = np.zeros(NW * 4, np.int64)
    groups = []   # (sb, ch, tile_off, n_tiles)
    # per-slot window id (for tile->window mapping), -1 = group pad
    slot_w_list = []
    so = 0
    for sb in range(NSB):
        ws = range(sb * SBW, min((sb + 1) * SBW, NW))
        for c4 in range(4):
            g_so = so
            for w_ in ws:
                cid = w_ * 4 + c4
                cell_slot_off[cid] = so
                slot_w_list.append(np.full(int(cap[cid]), w_, np.int64))
                so += int(cap[cid])
            g_slots = so - g_so
            pad = (-g_slots) % 128
            if pad:
                slot_w_list.append(np.full(pad, -1, np.int64))
                so += pad
            groups.append((sb, c4, g_so // 128, (so - g_so) // 128))
    SLOTS = so
    NT = SLOTS // 128
    slot_w = np.concatenate(slot_w_list)

    # tiles: windows overlapped; MM list (tile, window, iota_k)
    first_w = np.empty(NT, np.int64)
    mm_list = []           # (t, w, k) in emission order
    for t in range(NT):
        ws_here = slot_w[t * 128:(t + 1) * 128]
        ws_u = np.unique(ws_here[ws_here >= 0])
        fw = int(ws_u[0]) if len(ws_u) else 0
        first_w[t] = fw
        for w_ in ws_u:
            k = int(w_ - fw)
            assert 0 <= k < 5
            mm_list.append((t, int(w_), k))
    # psum-bank accumulation flags over mm_list order
    NBK = _ceil(NW, 4)
    firstmm = np.full(NBK, -1, np.int64)
    lastmm = np.full(NBK, -1, np.int64)
    for i, (t, w_, k) in enumerate(mm_list):
        bk = w_ // 4
        if firstmm[bk] < 0:
            firstmm[bk] = i
        lastmm[bk] = i
    assert (firstmm >= 0).all(), "every psum bank needs at least one MM"
    assert len({w_ for (_, w_, _k) in mm_list}) == NW, \
        "every window needs at least one MM"
    # extras: one shifted-dstrel sel column per k>0 MM, in emission order
    ex_cols = []   # (t, k)
    mm_flags = []
    for i, (t, w_, k) in enumerate(mm_list):
        e = -1
        if k > 0:
            e = len(ex_cols)
            ex_cols.append((t, k))
        mm_flags.append((t, w_, k, i == firstmm[w_ // 4],
                         i == lastmm[w_ // 4], e))
    NEX = len(ex_cols)
    NEXP = _ceil(max(NEX, 1), SLAB) * SLAB

    # per-core slot placement
    order = np.lexsort((srow, cell, core))
    core_s = core[order]
    cell_s = cell[order]
    keyall = core_s * (NW * 4) + cell_s
    starts = np.r_[0, np.flatnonzero(np.diff(keyall)) + 1]
    gid = np.zeros(len(keyall), np.int64)
    gid[starts[1:]] = 1
    gid = np.cumsum(gid)
    pos = np.arange(len(keyall)) - starts[gid]
    slot = cell_slot_off[cell_s] + pos
    assert (pos < cap[cell_s]).all()

    gidx_all = np.zeros((NCORES, SLOTS), np.int16)
    dstrel_all = np.full((NCORES, SLOTS), -1.0, np.float32)
    gidx_all[core_s, slot] = srow[order].astype(np.int16)
    # dstrel relative to the containing tile's first window
    tile_of_slot = slot // 128
    dstrel_all[core_s, slot] = (dl[order] - first_w[tile_of_slot] * 128
                                ).astype(np.float32)
    assert (dstrel_all[core_s, slot] >= 0).all()

    # device layouts
    gidx_dev = np.tile(
        gidx_all.reshape(NCORES, SLOTS // 16, 16).transpose(0, 2, 1), (1, 8, 1)
    ).copy()                                               # [8, 128, SLOTS//16]
    dstrel_dev = dstrel_all.reshape(NCORES, NT, 128).transpose(0, 2, 1).copy()

    dstrel_ex_dev = np.full((NCORES, 128, NEXP), -1000.0, np.float32)
    for e, (t, k) in enumerate(ex_cols):
        dstrel_ex_dev[:, :, e] = dstrel_dev[:, :, t] - 128.0 * k

    batch = np.asarray(batch, np.int64)
    brel = np.full((NCORES, NPAD), -1.0, np.float32)
    for cc in range(NCORES):
        brel[cc, :NLOC] = batch[cc * NLOC:(cc + 1) * NLOC]
    batchrel_dev = brel.reshape(NCORES, NW, 128).transpose(0, 2, 1).copy()

    x = np.asarray(x, np.float32)
    xt_dev = np.zeros((NCORES, 128, NPAD), bf16)
    dinvT_dev = np.zeros((NCORES, 128, NPAD), bf16)
    for cc in range(NCORES):
        xl = x[cc * NLOC:(cc + 1) * NLOC]                  # [NLOC, 128]
        xt_dev[cc, :, :NLOC] = xl.T.astype(bf16)
        dv = np.zeros(NPAD, np.float32)
        dv[:NLOC] = dinv[cc * NLOC:(cc + 1) * NLOC]
        dinvT_dev[cc] = np.broadcast_to(dv.astype(bf16), (128, NPAD))

    meta = dict(N=N, NLOC=NLOC, NPAD=NPAD, NW=NW, NSB=NSB,
                NT=NT, SLOTS=SLOTS, groups=groups, mm_flags=mm_flags,
                CHR=CHR, NEX=NEX, NEXP=NEXP)
    data = dict(gidx=gidx_dev, dstrel=dstrel_dev, batchrel=batchrel_dev,
                xt=xt_dev, dinvt=dinvT_dev, dstrel_ex=dstrel_ex_dev)
    return meta, data


def _epilogue(nc, sb, ps, h, dinvT, bg, l, NW, epip, AT, Relu):
    """h[:, sb windows] = relu(h + dinv*agg + bg[l])"""
    b16 = mybir.dt.bfloat16
    ws0 = sb * SBW
    wn = min(SBW, NW - ws0)
    cs = slice(ws0 * 128, (ws0 + wn) * 128)
    u = epip.tile([128, wn * 128], b16, tag="u", bufs=2, name="u")
    nc.vector.tensor_tensor(out=u[:], in0=ps[:, :wn * 128],
                            in1=dinvT[:, cs], op=AT.mult)
    u2 = epip.tile([128, wn * 128], b16, tag="u2", bufs=2, name="u2")
    nc.vector.tensor_tensor(out=u2[:], in0=u[:], in1=h[:, cs], op=AT.add)
    nc.scalar.activation(out=h[:, cs], in_=u2[:], func=Relu, bias=bg[:, l:l + 1])


# --------------------------------------------------------------------------
# device program
# --------------------------------------------------------------------------
def _build(meta, L, ablate=()):
    ablate = set(ablate)
    f32 = mybir.dt.float32
    b16 = mybir.dt.bfloat16
    f16 = mybir.dt.float16
    fp8 = mybir.dt.float8e4
    i16 = mybir.dt.int16
    NPAD, NW, NSB = meta["NPAD"], meta["NW"], meta["NSB"]
    NT, SLOTS = meta["NT"], meta["SLOTS"]
    NEXP = meta["NEXP"]
    groups, mm_flags = meta["groups"], meta["mm_flags"]
    CHR = meta["CHR"]
    SBWE = min(SBW, NW)
    rg = [list(range(NCORES))]
    # mm_flags grouped per tile for emission
    mm_by_tile = {}
    for (t, w_, k, st_f, sp_f, e) in mm_flags:
        mm_by_tile.setdefault(t, []).append((w_, k, st_f, sp_f, e))

    nc = bacc.Bacc("TRN2", target_bir_lowering=False, debug=False,
                   num_devices=NCORES)
    d_xt = nc.dram_tensor("xt", [128, NPAD], b16, kind="ExternalInput")
    d_dinvt = nc.dram_tensor("dinvt", [128, NPAD], b16, kind="ExternalInput")
    d_gidx = nc.dram_tensor("gidx", [128, SLOTS // 16], i16, kind="ExternalInput")
    d_dstrel = nc.dram_tensor("dstrel", [128, NT], f16, kind="ExternalInput")
    d_batchrel = nc.dram_tensor("batchrel", [128, NW], f16, kind="ExternalInput")
    d_w0 = nc.dram_tensor("w0", [128, 128], b16, kind="ExternalInput")
    d_wg = nc.dram_tensor("wg", [L, 128, 128], b16, kind="ExternalInput")
    d_wc1 = nc.dram_tensor("wc1", [128, 128], b16, kind="ExternalInput")
    d_wc2 = nc.dram_tensor("wc2", [128, C], b16, kind="ExternalInput")
    d_b0 = nc.dram_tensor("b0", [128, 1], f32, kind="ExternalInput")
    d_bg = nc.dram_tensor("bg", [L, 128, 1], f32, kind="ExternalInput")
    d_bc1 = nc.dram_tensor("bc1", [128, 1], f32, kind="ExternalInput")
    d_bc2m = nc.dram_tensor("bc2m", [G, C], f32, kind="ExternalInput")
    d_stair = nc.dram_tensor("stair", [128, 128 * SLAB], f16,
                             kind="ExternalInput")
    d_dstrel_ex = nc.dram_tensor("dstrel_ex", [128, NEXP], f16,
                                 kind="ExternalInput")
    d_id128 = nc.dram_tensor("id128", [128, 128], b16, kind="ExternalInput")
    d_idg = nc.dram_tensor("idg", [G, G], b16, kind="ExternalInput")
    d_out = nc.dram_tensor("out", [G, C], f32, kind="ExternalOutput")

    ag_in = [nc.dram_tensor(f"ag_in{l}", [128, NW, 128], b16)
             for l in range(L)]
    xw_t = [nc.dram_tensor(f"xw_t{l}", [NCORES * 128, NW * 128], b16,
                           addr_space="Shared") for l in range(L)]
    pool_in = nc.dram_tensor("pool_in", [G, 128], f32)
    pool_out = nc.dram_tensor("pool_out", [NCORES * G, 128], f32,
                              addr_space="Shared")

    Relu = mybir.ActivationFunctionType.Relu
    Exp = mybir.ActivationFunctionType.Exp
    Copy = mybir.ActivationFunctionType.Copy
    AT = mybir.AluOpType

    with tile.TileContext(nc) as tc:
        with (
            tc.tile_pool(name="state", bufs=1) as state,
            tc.tile_pool(name="wpool", bufs=1) as wpool,
            tc.tile_pool(name="xin", bufs=3) as xinp,
            tc.tile_pool(name="xws", bufs=3) as xwsp,
            tc.tile_pool(name="gix", bufs=2) as gixp,
            tc.tile_pool(name="gbf", bufs=3) as gbfp,
            tc.tile_pool(name="sel", bufs=3) as selp,
            tc.tile_pool(name="epi", bufs=6) as epip,
            tc.tile_pool(name="psxw", bufs=2, space="PSUM") as psxw,
            tc.tile_pool(name="pstr", bufs=2, space="PSUM") as pstr,
            tc.tile_pool(name="pswin", bufs=2, space="PSUM") as pswin,
        ):
            # ---- persistent state + constants ----
            h = state.tile([128, NPAD], b16, tag="h")
            dinvT = state.tile([128, NPAD], b16, tag="dinvT")
            dstrel = state.tile([128, NT], f16, tag="dstrel")
            xwp = state.tile([128, NW, 128], b16, tag="xwp")

            w0 = wpool.tile([128, 128], b16, tag="w0")
            nc.sync.dma_start(w0[:], d_w0[:])
            b0 = wpool.tile([128, 1], f32, tag="b0")
            nc.sync.dma_start(b0[:], d_b0[:])
            wg = wpool.tile([128, L, 128], b16, tag="wg")
            nc.sync.dma_start(wg[:], d_wg.rearrange("l p f -> p l f"))
            nc.sync.dma_start(dinvT[:], d_dinvt[:])
            nc.sync.dma_start(dstrel[:], d_dstrel[:])
            wc1 = wpool.tile([128, 128], b16, tag="wc1")
            nc.sync.dma_start(wc1[:], d_wc1[:])
            wc2 = wpool.tile([128, C], b16, tag="wc2")
            nc.sync.dma_start(wc2[:], d_wc2[:])
            bg = wpool.tile([128, L], f32, tag="bg")
            nc.sync.dma_start(bg[:], d_bg.rearrange("l p o -> p (l o)"))
            bc1 = wpool.tile([128, 1], f32, tag="bc1")
            nc.sync.dma_start(bc1[:], d_bc1[:])
            bc2m = wpool.tile([G, C], f32, tag="bc2m")
            nc.sync.dma_start(bc2m[:], d_bc2m[:])
            stair = wpool.tile([128, 128 * SLAB], f16, tag="stair")
            nc.sync.dma_start(stair[:], d_stair[:])
            dstrel_ex = wpool.tile([128, NEXP], f16, tag="dstrel_ex")
            nc.sync.dma_start(dstrel_ex[:], d_dstrel_ex[:])
            id128 = wpool.tile([128, 128], b16, tag="id128")
            nc.sync.dma_start(id128[:], d_id128[:])
            idg = wpool.tile([G, G], b16, tag="idg")
            nc.sync.dma_start(idg[:], d_idg[:])
            batchrel = wpool.tile([128, NW], f16, tag="batchrel")
            nc.sync.dma_start(batchrel[:], d_batchrel[:])
            nchunks = _ceil(NPAD, 512)

            def emit_phaseA_cols(l, c0, c1hi):
                """xw-table columns [c0, c1hi) for layer l from current h:
                compute, transpose to node-major, DMA into ag_in[l]."""
                while c0 < c1hi:
                    cw = min(512, c1hi - c0)
                    ps = psxw.tile([128, cw], f32, tag="psxw", name="ps")
                    nc.tensor.matmul(ps[:], lhsT=wg[:, l, :],
                                     rhs=h[:, c0:c0 + cw],
                                     start=True, stop=True)
                    xws = xwsp.tile([128, cw], b16, tag="xws", name="xws")
                    nc.vector.tensor_tensor(out=xws[:], in0=ps[:],
                                         in1=dinvT[:, c0:c0 + cw],
                                         op=AT.mult)
                    for j in range(cw // 128):
                        a = (c0 + j * 128) // 128
                        pst = pstr.tile([128, 128], b16, tag="pstr",
                                        name="pst")
                        nc.tensor.transpose(
                            pst[:], xws[:, j * 128:(j + 1) * 128],
                            id128[:])
                        nc.scalar.activation(out=xwp[:, a, :],
                                             in_=pst[:], func=Copy)
                    a0c = c0 // 128
                    a1c = (c0 + cw) // 128
                    nc.sync.dma_start(ag_in[l][:, a0c:a1c, :],
                                      xwp[:, a0c:a1c, :])
                    c0 += cw

            def emit_AG(l):
                if "noag" in ablate:
                    nc.sync.dma_start(
                        bass.AP(xw_t[l], 0, ag_in[l][:].ap), ag_in[l][:])
                else:
                    nc.gpsimd.collective_compute(
                        "AllGather", AT.bypass, ins=[ag_in[l][:]],
                        outs=[xw_t[l][:]], replica_groups=rg)

            import os as _os
            for _krep in range(int(_os.environ.get("BENCH_KREP", "1"))):
                # ---- stage 1: h = relu(W0.T @ xT + b0) ----
                for k in range(nchunks):
                    c0 = k * 512
                    cw = min(512, NPAD - c0)
                    xts = xinp.tile([128, cw], b16, tag="xts", name="xts")
                    nc.sync.dma_start(xts[:], d_xt[:, c0:c0 + cw])
                    ps = psxw.tile([128, cw], f32, tag="psxw", name="ps")
                    nc.tensor.matmul(ps[:], lhsT=w0[:], rhs=xts[:],
                                     start=True, stop=True)
                    nc.scalar.activation(out=h[:, c0:c0 + cw], in_=ps[:],
                                         func=Relu, bias=b0[:])
                emit_phaseA_cols(0, 0, NPAD)
                emit_AG(0)

                # ---- GCN layers ----
                for l in range(L):
                    # phase B: gather + segment-sum into per-window psum
                    ps_sb = {}
                    sel_tiles = {}
                    ex_tiles = {}

                    def get_sel(t):
                        s = t // SLAB
                        if s not in sel_tiles:
                            t0 = s * SLAB
                            tn = min(SLAB, NT - t0)
                            st = selp.tile([128, 128, tn], f16, tag="sel",
                                           name="st")
                            in0 = bass.AP(dstrel.tensor,
                                          dstrel[:, t0:t0 + tn].offset,
                                          [dstrel[:].ap[0], [0, 128], [1, tn]])
                            in1 = bass.AP(stair.tensor, stair[:].offset,
                                          [stair[:].ap[0], [SLAB, 128], [1, tn]])
                            nc.vector.tensor_tensor(out=st[:], in0=in0, in1=in1,
                                                    op=AT.is_equal)
                            sel_tiles.clear()
                            sel_tiles[s] = (st, tn)
                        st, tn = sel_tiles[s]
                        return st, t - s * SLAB, tn

                    def get_ex(e):
                        s = e // SLAB
                        if s not in ex_tiles:
                            e0 = s * SLAB
                            en = min(SLAB, NEXP - e0)
                            sx = selp.tile([128, 128, en], f16, tag="selx",
                                           name="sx", bufs=2)
                            in0 = bass.AP(dstrel_ex.tensor,
                                          dstrel_ex[:, e0:e0 + en].offset,
                                          [dstrel_ex[:].ap[0], [0, 128], [1, en]])
                            in1 = bass.AP(stair.tensor, stair[:].offset,
                                          [stair[:].ap[0], [SLAB, 128], [1, en]])
                            nc.vector.tensor_tensor(out=sx[:], in0=in0, in1=in1,
                                                    op=AT.is_equal)
                            ex_tiles.clear()
                            ex_tiles[s] = (sx, en)
                        sx, en = ex_tiles[s]
                        return sx, e - s * SLAB, en

                    # last layer: fold global-add-pool into the epilogues so
                    # the pool matmuls overlap the remaining phase B work
                    pool_st = {}
                    if l == L - 1:
                        pool_st["psp"] = psxw.tile([G, 128], f32, tag="psxw",
                                                   name="psp")

                    def emit_pool(sb_done):
                        for a in range(sb_done * SBW,
                                       min((sb_done + 1) * SBW, NW)):
                            pst = pstr.tile([128, 128], b16, tag="pstr",
                                            name="pst2")
                            nc.tensor.transpose(
                                pst[:], h[:, a * 128:(a + 1) * 128], id128[:])
                            hn = epip.tile([128, 128], b16, tag="hn",
                                           name="hn")
                            nc.scalar.activation(out=hn[:], in_=pst[:],
                                                 func=Copy)
                            if a % SLAB == 0:
                                a0 = a
                                an = min(SLAB, NW - a0)
                                bsel = selp.tile([128, G, an], f16, tag="sel",
                                                 name="bsel")
                                in0 = bass.AP(
                                    batchrel.tensor,
                                    batchrel[:, a0:a0 + an].offset,
                                    [batchrel[:].ap[0], [0, G], [1, an]])
                                in1 = bass.AP(
                                    stair.tensor, stair[:].offset,
                                    [stair[:].ap[0], [SLAB, G], [1, an]])
                                nc.vector.tensor_tensor(
                                    out=bsel[:], in0=in0, in1=in1,
                                    op=AT.is_equal)
                                pool_st["bsel"] = (bsel, a0, an)
                            bsel, a0, an = pool_st["bsel"]
                            blhs = bass.AP(bsel.tensor,
                                           bsel[:].offset + (a - a0),
                                           [bsel[:].ap[0], [an, G]])
                            nc.tensor.matmul(pool_st["psp"][:], lhsT=blhs,
                                             rhs=hn[:], start=(a == 0),
                                             stop=(a == NW - 1))

                    for (sb, c4, g_off, g_nt) in groups:
                        if sb not in ps_sb:
                            ps_sb.clear()
                            ps_sb[sb] = pswin.tile([128, SBWE * 128], f32,
                                                   name="pswin_t", tag="pswin")
                        if g_nt > 0:
                            slots = g_nt * 128
                            gb = gbfp.tile([128, g_nt, 128], b16, tag="gbf")
                            gixt = gixp.tile([128, slots // 16], i16, tag="gix")
                            so = g_off * 128
                            nc.sync.dma_start(
                                gixt[:], d_gidx[:, so // 16:(so + slots) // 16])
                            chv = bass.AP(xw_t[l], c4 * 128,
                                          [[4 * 128, CHR], [1, 128]])
                            nc.gpsimd.dma_gather(
                                gb[:], chv, gixt[:], slots, slots,
                                128, elem_step=4 * 128, single_packet=False)
                            for ti in range(g_nt):
                                t = g_off + ti
                                st, si, tn = get_sel(t)
                                for (w_, k, st_f, sp_f, e) in mm_by_tile.get(t, []):
                                    wr = w_ - sb * SBW
                                    if k == 0:
                                        rhs = bass.AP(
                                            st.tensor, st[:].offset + si,
                                            [st[:].ap[0], [tn, 128]])
                                    else:
                                        sx, se, en = get_ex(e)
                                        rhs = bass.AP(
                                            sx.tensor, sx[:].offset + se,
                                            [sx[:].ap[0], [en, 128]])
                                    nc.tensor.matmul(
                                        ps_sb[sb][:, wr * 128:(wr + 1) * 128],
                                        lhsT=gb[:, ti, :], rhs=rhs,
                                        start=bool(st_f), stop=bool(sp_f))
                        if c4 == 3:
                            _epilogue(nc, sb, ps_sb[sb], h, dinvT, bg, l, NW,
                                      epip, AT, Relu)
                            if l + 1 < L:
                                if sb == NSB - 1:
                                    emit_phaseA_cols(l + 1, 0, NPAD)
                                    emit_AG(l + 1)
                            else:
                                emit_pool(sb)

                # ---- global add pool: AllGather partials (cheaper than
                # AllReduce in the collective model), sum locally ----
                pool_sb = epip.tile([G, 128], f32, tag="poolsb")
                nc.vector.tensor_copy(out=pool_sb[:], in_=pool_st["psp"][:])
                nc.sync.dma_start(pool_in[:], pool_sb[:])
                if "noar" in ablate:
                    nc.sync.dma_start(
                        bass.AP(pool_out, 0, pool_in[:].ap), pool_in[:])
                else:
                    nc.gpsimd.collective_compute(
                        "AllGather", AT.bypass, ins=[pool_in[:]],
                        outs=[pool_out[:]], replica_groups=rg)

                # ---- classifier (replicated) ----
                # load partials as [G, 128 feat, 8 cores], reduce innermost
                pooled_a = epip.tile([G, 128, NCORES], f32, tag="pooleda",
                                     name="pooled_a")
                nc.sync.dma_start(
                    pooled_a[:],
                    bass.AP(pool_out, 0,
                            [[128, G], [1, 128], [G * 128, NCORES]]))
                pooled_f = epip.tile([G, 128], f32, tag="pooledf")
                nc.vector.tensor_reduce(
                    out=bass.AP(pooled_f.tensor, pooled_f[:].offset,
                                [pooled_f[:].ap[0], [1, 128], [0, 1]]),
                    in_=pooled_a[:], axis=mybir.AxisListType.X, op=AT.add)
                pooled_b = epip.tile([G, 128], b16, tag="pooledb")
                nc.vector.tensor_copy(out=pooled_b[:], in_=pooled_f[:])
                pstp = pstr.tile([128, G], b16, tag="pstr")
                nc.tensor.transpose(pstp[:], pooled_b[:], idg[:])
                pooledT = epip.tile([128, G], b16, tag="pooledT")
                nc.vector.tensor_copy(out=pooledT[:], in_=pstp[:])
                psz = pstr.tile([128, G], f32, tag="pstr")
                nc.tensor.matmul(psz[:], lhsT=wc1[:], rhs=pooledT[:],
                                 start=True, stop=True)
                zt = epip.tile([128, G], b16, tag="zt")
                nc.scalar.activation(out=zt[:], in_=psz[:], func=Relu, bias=bc1[:])
                pslg = pstr.tile([G, C], f32, tag="pstr")
                nc.tensor.matmul(pslg[:], lhsT=zt[:], rhs=wc2[:],
                                 start=True, stop=True)
                lg = epip.tile([G, C], f32, tag="lg")
                nc.vector.tensor_tensor(out=lg[:], in0=pslg[:], in1=bc2m[:], op=AT.add)
                # softmax over C (free dim)
                mx = epip.tile([G, 1], f32, tag="mx")
                nc.vector.tensor_reduce(out=mx[:], in_=lg[:],
                                        axis=mybir.AxisListType.X, op=AT.max)
                nmx = epip.tile([G, 1], f32, tag="nmx")
                nc.vector.tensor_scalar_mul(nmx[:], mx[:], -1.0)
                ex = epip.tile([G, C], f32, tag="ex")
                nc.scalar.activation(out=ex[:], in_=lg[:], func=Exp, bias=nmx[:])
                sm = epip.tile([G, 1], f32, tag="sm")
                nc.vector.tensor_reduce(out=sm[:], in_=ex[:],
                                        axis=mybir.AxisListType.X, op=AT.add)
                rs = epip.tile([G, 1], f32, tag="rs")
                nc.vector.reciprocal(rs[:], sm[:])
                prob = epip.tile([G, C], f32, tag="prob")
                nc.vector.tensor_scalar_mul(prob[:], ex[:], rs[:])
                nc.sync.dma_start(d_out[:], prob[:])

    nc.compile()
    return nc


def _ninsts(nc):
    return sum(len(b.instructions) for b in nc.m.functions[0].blocks)


# --------------------------------------------------------------------------
# entry point
# --------------------------------------------------------------------------
def kernel(x, edge_index, batch, W0, b0, Wg, bg, Wc1, bc1, Wc2, bc2,
           **extra):
    x = np.asarray(x, np.float32)
    edge_index = np.asarray(edge_index)
    batch = np.asarray(batch)
    W0 = np.asarray(W0, np.float32)
    Wg = np.asarray(Wg, np.float32)
    L = Wg.shape[0]

    key = (x.shape, edge_index.shape,
           hash(edge_index.tobytes()), hash(np.asarray(batch).tobytes()))
    if key not in _cache:
        meta, data = _preprocess(x, edge_index, batch)
        nc = _build(meta, L)
        _cache.clear()
        _cache[key] = (meta, data, nc)
    meta, data, nc = _cache[key]

    stair = np.broadcast_to(
        np.repeat(np.arange(128, dtype=np.float16), 32), (128, 128 * 32)).copy()
    common = dict(
        w0=W0.astype(bf16).view(np.uint16),
        wg=Wg.astype(bf16).view(np.uint16),
        wc1=np.asarray(Wc1, np.float32).astype(bf16).view(np.uint16),
        wc2=np.asarray(Wc2, np.float32).astype(bf16).view(np.uint16),
        b0=np.asarray(b0, np.float32).reshape(128, 1),
        bg=np.asarray(bg, np.float32).reshape(L, 128, 1),
        bc1=np.asarray(bc1, np.float32).reshape(128, 1),
        bc2m=np.broadcast_to(np.asarray(bc2, np.float32), (G, C)).copy(),
        stair=stair,
        id128=np.eye(128, dtype=np.float32).astype(bf16).view(np.uint16),
        idg=np.eye(G, dtype=np.float32).astype(bf16).view(np.uint16),
    )
    in_maps = []
    for c in range(NCORES):
        m = dict(common)
        m["xt"] = data["xt"][c].view(np.uint16)
        m["dinvt"] = data["dinvt"][c].view(np.uint16)
        m["gidx"] = data["gidx"][c]
        m["dstrel"] = data["dstrel"][c].astype(np.float16)
        m["dstrel_ex"] = data["dstrel_ex"][c].astype(np.float16)
        m["batchrel"] = data["batchrel"][c].astype(np.float16)
        in_maps.append(m)

    import os
    trace = os.environ.get("BASS_KERNEL_TRACE", "0") == "1"
    res = run_bass_kernel_spmd(nc, in_maps, list(range(NCORES)), trace=trace)
    kernel._last_exec_ns = res.exec_time_ns
    kernel._last_results = res
    return np.asarray(res.results[0]["out"], np.float32)


kernel._last_exec_ns = None

